# revision 15
# baseline (speedup 1.0000x reference)
"""Trainium2 Bass kernel for GAP -> tiny Mamba (channel attention) -> broadcast multiply.

Reference computation (per batch):
    pooled = mean(x1 over H,W)                  # [C] ; sequence of length C=512, d_model=1
    att    = mamba(pooled)                      # d_inner=2, d_state=16, dt_rank=1, conv=4
    out    = x2 * att[None, None, :]

Sharding: data-parallel over batch B=16 across 8 cores (2 batches/core), params
replicated. Memory-bound: each core streams 16 MiB of x1 (GAP), 16 MiB of x2 in
and 16 MiB of out back (~48 MiB -> ~140 us roofline at ~358 GB/s HBM/core).

Implementation notes:
  - GAP: x1 streamed as [128, 8*512] tiles (16 KiB contiguous per partition),
    accumulated on VectorE into [128, C] then reduced across partitions with a
    ones-vector matmul into PSUM.
  - The length-512 selective scan runs as ONE VectorE tensor_tensor_scan
    instruction over a [64 (b,d,s) lanes, 512 (t)] layout:
        h[:, t] = dA[:, t] * h[:, t-1] + dBu[:, t]
  - All tiny projections (in_proj/x_proj/dt_proj/out_proj, s-broadcasts and the
    s-reduction) are TensorE matmuls with small selector matrices; the
    value-carrying selectors are scattered from the runtime weight tensors with
    tiny DMAs, the 0/1 patterns are inline (NEFF-const) tensors.
  - Phase 2 multiplies x2 tiles in place by the per-batch attention row
    (broadcast to 128 partitions via a ones-matmul) and streams them out.
"""

import os
import numpy as np

import concourse.bass as bass
import concourse.bacc as bacc
import concourse.tile as tile
from concourse import mybir
from concourse.bass_utils import run_bass_kernel_spmd

F32 = mybir.dt.float32
AF = mybir.ActivationFunctionType
OP = mybir.AluOpType

N_CORES = 8
B_FULL, H, W, C = 16, 64, 64, 512
B_LOC = B_FULL // N_CORES            # 2 batches per core
HW = H * W                           # 4096 spatial positions
Q = 8                                # image rows per partition per stream tile
ROWS_PER_TILE = 128 * Q              # 1024
N_TILES = HW // ROWS_PER_TILE        # 4 tiles per batch image

WEIGHT_SHAPES = {
    "in_proj_w": [4, 1],
    "conv_w": [2, 1, 4],
    "conv_b": [2],
    "x_proj_w": [33, 2],
    "dt_proj_w": [2, 1],
    "dt_proj_b": [2],
    "A_log": [2, 16],
    "Dp": [2],
    "out_proj_w": [1, 2],
}

LAST_RESULTS = None
_CACHE = {}


def _dap(handle, offset, pattern):
    return bass.AP(handle, offset, pattern)


def _build():
    # Bacc (not raw Bass): its compile() pipeline legalizes multi-wait
    # instructions, which this walrus version rejects on e.g. TensorTensor.
    nc = bacc.Bacc(None, target_bir_lowering=False)

    x1h = nc.dram_tensor("x1", [B_LOC, H, W, C], F32, kind="ExternalInput")
    x2h = nc.dram_tensor("x2", [B_LOC, H, W, C], F32, kind="ExternalInput")
    wh = {
        name: nc.dram_tensor(name, shape, F32, kind="ExternalInput")
        for name, shape in WEIGHT_SHAPES.items()
    }
    outh = nc.dram_tensor("out", [B_LOC, H, W, C], F32, kind="ExternalOutput")

    # ---- inline 0/1 constants ----
    # GAP reduction vector with the mean folded in
    ones_col_d = nc.inline_tensor(np.full((128, 1), 1.0 / HW, np.float32), "c_ones_col")
    # row broadcast selectors u[b] -> rows (b,d):  [1, 4] each
    selu_np = [np.zeros((1, 4), np.float32) for _ in range(2)]
    for b in range(2):
        selu_np[b][0, 2 * b : 2 * b + 2] = 1.0
    selu_d = [nc.inline_tensor(selu_np[b], f"c_selu{b}") for b in range(2)]
    # (b,d) -> (b,d,s) broadcast selector
    bsel_np = np.zeros((4, 64), np.float32)
    for r in range(4):
        bsel_np[r, 16 * r : 16 * r + 16] = 1.0
    bsel_d = nc.inline_tensor(bsel_np, "c_bsel")
    # (b,d,s) -> (b,d) reduction selector
    rsel_np = np.zeros((64, 4), np.float32)
    for r in range(64):
        rsel_np[r, r // 16] = 1.0
    rsel_d = nc.inline_tensor(rsel_np, "c_rsel")
    ones128_d = nc.inline_tensor(np.ones((1, 128), np.float32), "c_ones128")
    # extraction selectors from xdbl rows (33b + r) of the x_proj output:
    #   dtr -> rows (b,d), B/C -> rows (b,d,s)
    selD_np = np.zeros((66, 4), np.float32)
    selB_np = np.zeros((66, 64), np.float32)
    selC_np = np.zeros((66, 64), np.float32)
    for b in range(2):
        for d in range(2):
            selD_np[33 * b, 2 * b + d] = 1.0
            for s in range(16):
                selB_np[33 * b + 1 + s, b * 32 + d * 16 + s] = 1.0
                selC_np[33 * b + 17 + s, b * 32 + d * 16 + s] = 1.0
    selD_d = nc.inline_tensor(selD_np, "c_selD")
    selB_d = nc.inline_tensor(selB_np, "c_selB")
    selC_d = nc.inline_tensor(selC_np, "c_selC")

    def img_ap(handle, b, t):
        # [128, Q*C] view of image rows [t*1024, (t+1)*1024) of batch b:
        # partition p holds Q consecutive rows (Q*C contiguous floats).
        off = (b * HW + t * ROWS_PER_TILE) * C
        return _dap(handle, off, [[Q * C, 128], [1, Q * C]])

    with tile.TileContext(nc) as tc:
        with (
            tc.tile_pool(name="x1pool", bufs=3) as x1pool,
            tc.tile_pool(name="x2pool", bufs=4) as x2pool,
            tc.tile_pool(name="work", bufs=1) as work,
            tc.tile_pool(name="psum", bufs=8, space="PSUM") as psum,
        ):
            # ================= setup: weight-derived selector tiles ========
            ones_col = work.tile([128, 1], F32)
            nc.gpsimd.dma_start(out=ones_col[:], in_=ones_col_d.ap())
            selu = []
            for b in range(2):
                su = work.tile([1, 4], F32, tag=f"selu{b}")
                nc.gpsimd.dma_start(out=su[:], in_=selu_d[b].ap())
                selu.append(su)
            bsel = work.tile([4, 64], F32)
            nc.gpsimd.dma_start(out=bsel[:], in_=bsel_d.ap())
            rsel = work.tile([64, 4], F32)
            nc.gpsimd.dma_start(out=rsel[:], in_=rsel_d.ap())
            ones128 = work.tile([1, 128], F32)
            nc.gpsimd.dma_start(out=ones128[:], in_=ones128_d.ap())
            selD = work.tile([66, 4], F32)
            nc.gpsimd.dma_start(out=selD[:], in_=selD_d.ap())
            selB = work.tile([66, 64], F32)
            nc.gpsimd.dma_start(out=selB[:], in_=selB_d.ap())
            selC = work.tile([66, 64], F32)
            nc.gpsimd.dma_start(out=selC[:], in_=selC_d.ap())

            # per-(b,d) scalar columns, rows ordered r = 2*b + d
            winx_col = work.tile([4, 1], F32)   # in_proj_w[d, 0]
            wz_col = work.tile([4, 1], F32)     # in_proj_w[2+d, 0]
            convb_col = work.tile([4, 1], F32)  # conv_b[d]
            dtw_col = work.tile([4, 1], F32)    # dt_proj_w[d, 0]
            dtb_col = work.tile([4, 1], F32)    # dt_proj_b[d]
            dp_col = work.tile([4, 1], F32)     # Dp[d]
            for b in range(2):
                sl = slice(2 * b, 2 * b + 2)
                nc.gpsimd.dma_start(out=winx_col[sl, :], in_=_dap(wh["in_proj_w"], 0, [[1, 2], [1, 1]]))
                nc.gpsimd.dma_start(out=wz_col[sl, :], in_=_dap(wh["in_proj_w"], 2, [[1, 2], [1, 1]]))
                nc.gpsimd.dma_start(out=convb_col[sl, :], in_=_dap(wh["conv_b"], 0, [[1, 2], [1, 1]]))
                nc.gpsimd.dma_start(out=dtw_col[sl, :], in_=_dap(wh["dt_proj_w"], 0, [[1, 2], [1, 1]]))
                nc.gpsimd.dma_start(out=dtb_col[sl, :], in_=_dap(wh["dt_proj_b"], 0, [[1, 2], [1, 1]]))
                nc.gpsimd.dma_start(out=dp_col[sl, :], in_=_dap(wh["Dp"], 0, [[1, 2], [1, 1]]))

            # conv taps with in_proj weight folded in: wq[r, j] = w_in[d]*conv_w[d,0,j]
            wq = work.tile([4, 4], F32)
            for b in range(2):
                for d in range(2):
                    nc.gpsimd.dma_start(
                        out=wq[2 * b + d : 2 * b + d + 1, :],
                        in_=_dap(wh["conv_w"], 4 * d, [[0, 1], [1, 4]]),
                    )
            nc.vector.tensor_scalar_mul(wq[:], wq[:], winx_col[:])

            # block-diagonal x_proj selector: xpw66[2b+d, 33b+r] = x_proj_w[r, d]
            xpw66 = work.tile([4, 66], F32)
            nc.vector.memset(xpw66[:], 0.0)
            for b in range(2):
                for d in range(2):
                    nc.gpsimd.dma_start(
                        out=xpw66[2 * b + d : 2 * b + d + 1, 33 * b : 33 * b + 33],
                        in_=_dap(wh["x_proj_w"], d, [[0, 1], [2, 33]]),
                    )

            # out_proj selector [4, 2]: osel[2b+d, b] = out_proj_w[0, d]
            osel = work.tile([4, 2], F32)
            nc.vector.memset(osel[:], 0.0)
            for b in range(2):
                nc.gpsimd.dma_start(
                    out=osel[2 * b : 2 * b + 2, b : b + 1],
                    in_=_dap(wh["out_proj_w"], 0, [[1, 2], [1, 1]]),
                )

            # A column [64, 1]: rows (b,d,s) = -exp(A_log[d, s])
            a_col = work.tile([64, 1], F32)
            for b in range(2):
                for d in range(2):
                    base = b * 32 + d * 16
                    nc.gpsimd.dma_start(
                        out=a_col[base : base + 16, :],
                        in_=_dap(wh["A_log"], 16 * d, [[1, 16], [1, 1]]),
                    )
            nc.scalar.activation(a_col[:], a_col[:], AF.Exp)
            nc.scalar.mul(a_col[:], a_col[:], -1.0)

            # ================= phase 1: stream x1, GAP ====================
            accs = []
            for b in range(2):
                acc = work.tile([128, C], F32, tag=f"acc{b}")
                nc.vector.memset(acc[:], 0.0)
                accs.append(acc)
            for b in range(2):
                for t in range(N_TILES):
                    x1t = x1pool.tile([128, Q * C], F32, tag="x1t")
                    nc.sync.dma_start(out=x1t[:], in_=img_ap(x1h, b, t))
                    v = x1t.rearrange("p (q c) -> p q c", q=Q)
                    for qi in range(Q):
                        nc.vector.tensor_add(accs[b][:], accs[b][:], v[:, qi, :])

            gaps = []
            u1 = []
            for b in range(2):
                gp = psum.tile([1, C], F32, tag="pp")
                nc.tensor.matmul(gp[:], ones_col[:], accs[b][:], start=True, stop=True)
                gaps.append(gp)
                u1b = work.tile([1, C], F32, tag=f"u1_{b}")
                nc.vector.tensor_copy(u1b[:], gp[:])
                u1.append(u1b)

            # ================= small mamba pipeline =======================
            # broadcast u to rows (b,d): two accumulating matmuls
            ubc = psum.tile([4, C], F32, tag="pp")
            for b in range(2):
                nc.tensor.matmul(ubc[:], selu[b][:], u1[b][:], start=(b == 0), stop=(b == 1))

            # causal depthwise conv (kernel 4) with folded input projection
            acc4 = work.tile([4, C], F32)
            nc.vector.tensor_scalar_mul(acc4[:], ubc[:], wq[:, 3:4])
            for j in (2, 1, 0):
                s = 3 - j
                nc.vector.scalar_tensor_tensor(
                    acc4[:, s:C], ubc[:, 0 : C - s], wq[:, j : j + 1], acc4[:, s:C],
                    op0=OP.mult, op1=OP.add,
                )
            # xconv = silu(acc4 + conv_b) composed as x*sigmoid(x)
            pre4 = work.tile([4, C], F32)
            nc.vector.tensor_scalar_add(pre4[:], acc4[:], convb_col[:])
            xsig4 = work.tile([4, C], F32)
            nc.scalar.activation(xsig4[:], pre4[:], AF.Sigmoid)
            xconv4 = work.tile([4, C], F32)
            nc.vector.tensor_mul(xconv4[:], pre4[:], xsig4[:])
            # silu(z) with z = u * w_in[2+d]
            zpre4 = work.tile([4, C], F32)
            nc.vector.tensor_scalar_mul(zpre4[:], ubc[:], wz_col[:])
            zsig4 = work.tile([4, C], F32)
            nc.scalar.activation(zsig4[:], zpre4[:], AF.Sigmoid)
            sz4 = work.tile([4, C], F32)
            nc.vector.tensor_mul(sz4[:], zpre4[:], zsig4[:])

            # x_proj: xdbl66 rows 33b+r = (x_proj_w @ xconv[b])[r]  -> one matmul
            xdbl66 = psum.tile([66, C], F32, tag="pp")
            nc.tensor.matmul(xdbl66[:], xpw66[:], xconv4[:], start=True, stop=True)
            xdbl_s = work.tile([66, C], F32)
            nc.vector.tensor_copy(xdbl_s[:], xdbl66[:])

            # dt = softplus(dtr * dt_proj_w + dt_proj_b) on rows (b,d)
            dtrbc = psum.tile([4, C], F32, tag="pp")
            nc.tensor.matmul(dtrbc[:], selD[:], xdbl_s[:], start=True, stop=True)
            # softplus(x) = ln(1 + exp(x)); |x| is tiny here so this is safe
            dtpre = work.tile([4, C], F32)
            nc.vector.tensor_scalar(
                dtpre[:], dtrbc[:], dtw_col[:], dtb_col[:], op0=OP.mult, op1=OP.add
            )
            dte = work.tile([4, C], F32)
            nc.scalar.activation(dte[:], dtpre[:], AF.Exp)
            nc.vector.tensor_scalar_add(dte[:], dte[:], 1.0)
            dt4 = work.tile([4, C], F32)
            nc.scalar.activation(dt4[:], dte[:], AF.Ln)

            # B/C rows replicated over d: rows (b,d,s)
            bm64p = psum.tile([64, C], F32, tag="pp")
            nc.tensor.matmul(bm64p[:], selB[:], xdbl_s[:], start=True, stop=True)
            bm64 = work.tile([64, C], F32)
            nc.vector.tensor_copy(bm64[:], bm64p[:])
            cm64p = psum.tile([64, C], F32, tag="pp")
            nc.tensor.matmul(cm64p[:], selC[:], xdbl_s[:], start=True, stop=True)

            # dA = exp(dt * A), dBu = dt * x * B  on 64 lanes
            g4 = work.tile([4, C], F32)
            nc.vector.tensor_mul(g4[:], dt4[:], xconv4[:])
            dt64p = psum.tile([64, C], F32, tag="pp")
            nc.tensor.matmul(dt64p[:], bsel[:], dt4[:], start=True, stop=True)
            g64p = psum.tile([64, C], F32, tag="pp")
            nc.tensor.matmul(g64p[:], bsel[:], g4[:], start=True, stop=True)
            da64 = work.tile([64, C], F32)
            nc.scalar.activation(da64[:], dt64p[:], AF.Exp, scale=a_col[:])
            dbu64 = work.tile([64, C], F32)
            nc.vector.tensor_mul(dbu64[:], g64p[:], bm64[:])

            # selective scan: h[:, t] = dA[:, t]*h[:, t-1] + dBu[:, t]
            h64 = work.tile([64, C], F32)
            nc.vector.tensor_tensor_scan(
                h64[:], da64[:], dbu64[:], 0.0, op0=OP.mult, op1=OP.add
            )

            # y = C . h (reduce s), + D*x, * silu(z), out_proj
            hc64 = work.tile([64, C], F32)
            nc.vector.tensor_mul(hc64[:], h64[:], cm64p[:])
            y4p = psum.tile([4, C], F32, tag="pp")
            nc.tensor.matmul(y4p[:], rsel[:], hc64[:], start=True, stop=True)
            y4g = work.tile([4, C], F32)
            nc.vector.scalar_tensor_tensor(
                y4g[:], xconv4[:], dp_col[:], y4p[:], op0=OP.mult, op1=OP.add
            )
            nc.vector.tensor_mul(y4g[:], y4g[:], sz4[:])
            # att[b] = sum_d out_proj_w[0,d] * y[b,d]  (one [1, C] psum per batch
            # so every compute AP sits at base partition 0), then broadcast to
            # 128 partitions via a ones matmul.
            att_bc = []
            for b in range(2):
                a1p = psum.tile([1, C], F32, tag="pp")
                nc.tensor.matmul(a1p[:], osel[:, b : b + 1], y4g[:], start=True, stop=True)
                a1 = work.tile([1, C], F32, tag=f"att1_{b}")
                nc.vector.tensor_copy(a1[:], a1p[:])
                abp = psum.tile([128, C], F32, tag="pp")
                nc.tensor.matmul(abp[:], ones128[:], a1[:], start=True, stop=True)
                ab = work.tile([128, C], F32, tag=f"attbc{b}")
                nc.vector.tensor_copy(ab[:], abp[:])
                att_bc.append(ab)

            # ================= phase 2: stream x2 * att -> out ============
            for b in range(2):
                for t in range(N_TILES):
                    x2t = x2pool.tile([128, Q * C], F32, tag="x2t")
                    nc.sync.dma_start(out=x2t[:], in_=img_ap(x2h, b, t))
                    v = x2t.rearrange("p (q c) -> p q c", q=Q)
                    for qi in range(Q):
                        nc.vector.tensor_mul(v[:, qi, :], v[:, qi, :], att_bc[b][:])
                    nc.sync.dma_start(out=img_ap(outh, b, t), in_=x2t[:])

    nc.compile()
    return nc


def _get_nc():
    if "nc" not in _CACHE:
        _CACHE["nc"] = _build()
    return _CACHE["nc"]


def kernel(**inputs):
    global LAST_RESULTS
    nc = _get_nc()
    ins = {k: np.ascontiguousarray(np.asarray(v, dtype=np.float32)) for k, v in inputs.items()}

    in_maps = []
    for i in range(N_CORES):
        m = {name: ins[name] for name in WEIGHT_SHAPES}
        m["x1"] = np.ascontiguousarray(ins["x1"][B_LOC * i : B_LOC * (i + 1)])
        m["x2"] = np.ascontiguousarray(ins["x2"][B_LOC * i : B_LOC * (i + 1)])
        in_maps.append(m)

    res = run_bass_kernel_spmd(
        nc,
        in_maps,
        core_ids=list(range(N_CORES)),
        trace=bool(int(os.environ.get("BASS_TRACE", "0") or "0")),
    )
    LAST_RESULTS = res
    return np.concatenate([r["out"] for r in res.results], axis=0)
